# revision 18
# baseline (speedup 1.0000x reference)
"""GAT cell (gnn_message_passing) Bass kernel for 8 Trainium2 NeuronCores.

Sharding: pure data parallelism over batch (64 graphs -> 8 per core), both
branches (in/out) on every core.

Host-side sharding also prepares layouts: bf16 cast (exact for the 0/1
adjacencies), row-chunking to the 128-partition grid, and the A^T / input^T
transposes, so the device does pure compute with large contiguous DMAs.

Math (per graph, per branch), done entirely in a TRANSPOSED layout so no
per-batch transposes of computed tensors are ever needed:
  x^T   = W_head^T @ input^T                      [att, N]
  xa^T  = a * x^T   (per-partition scale)
  s^T   = x @ (x*a)^T  via lhsT=x^T, rhs=xa^T     [N(j), N(i)]  == score^T
  B     = A^T;  B^k = (A^k)^T via lhsT=A (natural layout!)
  mask^T= binarize(B + B^2 + ... + B^order)       (exact in bf16: small ints)
  P^T   = exp(leakyrelu(s^T)) * mask^T            [j, i]
  Y     = input @ W_edge  via lhsT=input^T        [N(j), att]; augment ones col
  U     = P @ [Y | 1] via lhsT=P^T                [N(i), att+1]; col att = rowsum
  out   = U[:, :att] / (rowsum + eps) + bias
This equals softmax(where(mask, score, -1e12), axis=-1)*mask @ input @ W_edge
+ bias exactly (masked exps are exactly 0; all-masked rows give 0 rows).

PSUM bank trick for the reachability accumulator: B^2 matmuls write the bank,
the bank is evacuated to SBUF (rhs for B^3) while I@B re-adds and the B^3
matmuls keep accumulating into the same bank, so no separate I@B^2 pass.
"""

import numpy as np
from contextlib import ExitStack

import concourse.bass as bass
import concourse.bacc as bacc
import concourse.tile as tile
from concourse import mybir, bass_utils

F32, BF16 = mybir.dt.float32, mybir.dt.bfloat16
AF = mybir.ActivationFunctionType
ALU = mybir.AluOpType

NCORES = 8
B = 64
BPC = B // NCORES        # batches per core
N = 200                  # nodes per graph
H = 256                  # feature dim
ATT = 64                 # head dim
CH = [(0, 128), (1, 72)]  # (chunk index, rows) for the N=200 row split
EPS = 1e-20
BRS = ("in", "out")


def _make_identity(nc, identity):
    nc.gpsimd.memset(identity, 0.0)
    nc.gpsimd.affine_select(
        out=identity, in_=identity, compare_op=ALU.not_equal, fill=1.0,
        base=0, pattern=[[-1, 128]], channel_multiplier=1)


def _emit(ctx, tc, order, AN, AT, XT, WH, WE, AV, BV, O):
    nc = tc.nc
    consts = ctx.enter_context(tc.tile_pool(name="consts", bufs=1))
    pin = ctx.enter_context(tc.tile_pool(name="pin", bufs=6))
    pw = ctx.enter_context(tc.tile_pool(name="pw", bufs=4))
    pp1 = ctx.enter_context(tc.tile_pool(name="pp1", bufs=1, space="PSUM"))
    pp2 = ctx.enter_context(tc.tile_pool(name="pp2", bufs=2, space="PSUM"))

    ident = consts.tile([128, 128], BF16, tag="ident", name="ident")
    _make_identity(nc, ident)

    wh, we, av, bias = {}, {}, {}, {}
    for br in BRS:
        wh[br] = consts.tile([128, 2, ATT], BF16, tag=f"wh_{br}", name=f"wh_{br}")
        nc.sync.dma_start(out=wh[br], in_=WH[br])
        we[br] = consts.tile([128, 2, ATT], BF16, tag=f"we_{br}", name=f"we_{br}")
        nc.sync.dma_start(out=we[br], in_=WE[br])
        av[br] = consts.tile([128, 1], F32, tag=f"av_{br}", name=f"av_{br}")
        nc.sync.dma_start(out=av[br], in_=AV[br].rearrange("(a o) -> a o", o=1))
        bias[br] = consts.tile([128, ATT], F32, tag=f"bias_{br}", name=f"bias_{br}")
        bcast = bass.AP(tensor=BV[br].tensor, offset=BV[br].offset,
                        ap=[[0, 128], [1, ATT]])
        nc.gpsimd.dma_start(out=bias[br], in_=bcast)

    for br in BRS:
        for pb in range(0, BPC, 2):
            # ---- one packed load per batch pair ----
            buf = pin.tile([128, 2, 1424], BF16, tag="buf", name="buf")
            nc.sync.dma_start(out=buf, in_=AN[br][pb:pb + 2].rearrange(
                "b p f -> p b f"))
            a0_, T_, iT_ = [], [], []
            for i in range(2):
                a0_.append(buf[:, i, 0:512].rearrange("p (c m) -> p c m", c=2))
                T_.append(buf[:, i, 512:912].rearrange("p (c m) -> p c m", c=2))
                iT_.append(buf[:, i, 912:1424].rearrange("p (c m) -> p c m", c=2))

            # ---- x^T for both batches packed on 128 partitions ----
            # batch pb on partitions 0:64, batch pb+1 on 64:128; the W_head
            # stationary is shared, halving LDWEIGHTS.
            xt_ps = pp1.tile([128, 256], F32, tag="xt_ps", name="xt_ps")
            for i in range(2):
                for hc in range(2):
                    nc.tensor.matmul(xt_ps[i * 64:(i + 1) * 64, :],
                                     wh[br][:, hc, :], iT_[i][:, hc, :],
                                     start=(hc == 0), stop=(hc == 1))
            xt = pw.tile([128, 256], BF16, tag="xt", name="xt")
            nc.scalar.activation(out=xt, in_=xt_ps, func=AF.Copy)
            xa = pw.tile([128, 256], BF16, tag="xa", name="xa")
            nc.vector.tensor_scalar(out=xa, in0=xt, scalar1=av[br], scalar2=None,
                                    op0=ALU.mult)

            for i in range(2):
                b = pb + i
                a0, T, iT = a0_[i], T_[i], iT_[i]
                xtb = xt[i * 64:(i + 1) * 64, :]
                xab = xa[i * 64:(i + 1) * 64, :]

                # ---- score^T then exp(leaky(.)) ----
                sc_ps = pp2.tile([128, 2, N], F32, tag="sc_ps", name="sc_ps")
                for jc in range(2):
                    nc.tensor.matmul(sc_ps[:, jc, :],
                                     xtb[:, jc * 128:(jc + 1) * 128],
                                     xab[:, 0:N], start=True, stop=True)
                ls = pw.tile([128, 2, N], BF16, tag="ls", name="ls")
                nc.scalar.activation(out=ls, in_=sc_ps, func=AF.Prelu, alpha=0.2)
                es = pw.tile([128, 2, N], BF16, tag="es", name="es")
                nc.scalar.activation(out=es, in_=ls, func=AF.Exp)

                # ---- reachability: B^2 bank, then B^3 bank = I@B^2 + B^3 ----
                b23 = None
                if order == 2:
                    b23 = pp2.tile([128, 2, N], F32, tag="b23", name="b23")
                    for mc in range(2):
                        for kc in range(2):
                            nc.tensor.matmul(b23[:, mc, :],
                                             a0[:, kc, mc * 128:(mc + 1) * 128],
                                             T[:, kc, :],
                                             start=(kc == 0), stop=(kc == 1))
                elif order >= 3:
                    assert order == 3, "only order<=3 supported"
                    b2_ps = pp1.tile([128, 2, N], F32, tag="b2_ps", name="b2_ps")
                    for mc in range(2):
                        for kc in range(2):
                            nc.tensor.matmul(b2_ps[:, mc, :],
                                             a0[:, kc, mc * 128:(mc + 1) * 128],
                                             T[:, kc, :],
                                             start=(kc == 0), stop=(kc == 1))
                    b2 = pw.tile([128, 2, N], BF16, tag="b2", name="b2")
                    nc.scalar.activation(out=b2, in_=b2_ps, func=AF.Copy)
                    b23 = pp2.tile([128, 2, N], F32, tag="b23", name="b23")
                    for mc in range(2):
                        nc.tensor.matmul(b23[:, mc, :], ident, b2[:, mc, :],
                                         start=True, stop=False)
                        for kc in range(2):
                            nc.tensor.matmul(b23[:, mc, :],
                                             a0[:, kc, mc * 128:(mc + 1) * 128],
                                             b2[:, kc, :],
                                             start=False, stop=(kc == 1))

                # ---- P^T = exp(leaky(s^T)) * max(bin(B^2+..), B), padded ----
                pt = pw.tile([128, 2, 256], BF16, tag="pt", name="pt")
                nc.gpsimd.memset(pt[:, :, N:256], 0.0)
                if order >= 2:
                    mk = pw.tile([128, 2, N], BF16, tag="mk", name="mk")
                    nc.vector.scalar_tensor_tensor(
                        out=mk, in0=b23, scalar=0.0, in1=T,
                        op0=ALU.is_gt, op1=ALU.max)
                else:
                    mk = T
                nc.vector.tensor_tensor(out=pt[:, :, 0:N], in0=es, in1=mk,
                                        op=ALU.mult)

                # ---- Y = input @ W_edge (+ ones column) ----
                y_ps = pp1.tile([128, 2, ATT + 1], F32, tag="y_ps", name="y_ps")
                for jc in range(2):
                    for hc in range(2):
                        nc.tensor.matmul(y_ps[:, jc, 0:ATT],
                                         iT[:, hc, jc * 128:(jc + 1) * 128],
                                         we[br][:, hc, :],
                                         start=(hc == 0), stop=(hc == 1))
                ys = pw.tile([128, 2, ATT + 1], BF16, tag="ys", name="ys")
                nc.scalar.activation(out=ys[:, :, 0:ATT], in_=y_ps[:, :, 0:ATT],
                                     func=AF.Copy)
                nc.gpsimd.memset(ys[:, :, ATT:ATT + 1], 1.0)

                # ---- U = P @ [Y|1] ; normalize + bias ----
                o_ps = pp1.tile([128, 2, ATT + 1], F32, tag="o_ps", name="o_ps")
                for ic in range(2):
                    for jc in range(2):
                        nc.tensor.matmul(o_ps[:, ic, :],
                                         pt[:, jc, ic * 128:(ic + 1) * 128],
                                         ys[:, jc, :],
                                         start=(jc == 0), stop=(jc == 1))
                r = pw.tile([128, 2, 1], F32, tag="r", name="r")
                nc.vector.tensor_scalar(out=r, in0=o_ps[:, :, ATT:ATT + 1],
                                        scalar1=EPS, scalar2=None, op0=ALU.add)
                nc.vector.reciprocal(out=r, in_=r)
                res = pw.tile([128, 2, ATT], F32, tag="res", name="res")
                for ic in range(2):
                    nc.vector.scalar_tensor_tensor(out=res[:, ic, :],
                                                   in0=o_ps[:, ic, 0:ATT],
                                                   scalar=r[:, ic, 0:1],
                                                   in1=bias[br],
                                                   op0=ALU.mult, op1=ALU.add)
                nc.gpsimd.dma_start(
                    out=O[br][b].rearrange("(c p) d -> p c d", c=2), in_=res)


def build(order: int) -> bacc.Bacc:
    nc = bacc.Bacc("TRN2", target_bir_lowering=False, debug=False,
                   enable_asserts=True, num_devices=NCORES)
    AN, AT, XT, WH, WE, AV, BV, O = {}, {}, {}, {}, {}, {}, {}, {}
    for br in BRS:
        AN[br] = nc.dram_tensor(f"IN_{br}", [BPC, 128, 1424], BF16,
                                kind="ExternalInput").ap()
        AT[br] = None
        XT[br] = None
        WH[br] = nc.dram_tensor(f"WH_{br}", [128, 2, ATT], BF16,
                                kind="ExternalInput").ap()
        WE[br] = nc.dram_tensor(f"WE_{br}", [128, 2, ATT], BF16,
                                kind="ExternalInput").ap()
        AV[br] = nc.dram_tensor(f"AV_{br}", [128], F32, kind="ExternalInput").ap()
        BV[br] = nc.dram_tensor(f"BV_{br}", [ATT], F32, kind="ExternalInput").ap()
        O[br] = nc.dram_tensor(f"O_{br}", [BPC, 256, ATT], F32,
                               kind="ExternalOutput").ap()
    with tile.TileContext(nc) as tc:
        with ExitStack() as ctx:
            _emit(ctx, tc, order, AN, AT, XT, WH, WE, AV, BV, O)
    nc.compile()
    return nc


_CACHE = {}


def _get(order: int) -> bacc.Bacc:
    if order not in _CACHE:
        _CACHE[order] = build(order)
    return _CACHE[order]


def _bf16():
    import ml_dtypes
    return ml_dtypes.bfloat16


def _chunk_rows(x, pad_to=None):
    """[..., R, C] f32 -> [..., 128, 2, Cp] bf16: rows chunked to the
    128-partition grid (zero rows 72..127 of chunk 1 when R==200) and the
    free dim optionally zero-padded to ``pad_to``."""
    bf = _bf16()
    lead = x.shape[:-2]
    r, c = x.shape[-2:]
    cp = pad_to or c
    out = np.zeros(lead + (2, 128, cp), dtype=bf)
    xb = x.astype(bf)
    out[..., 0, 0:128, 0:c] = xb[..., 0:128, :]
    out[..., 1, 0:r - 128, 0:c] = xb[..., 128:r, :]
    # reorder to [..., 128, 2, Cp]
    return np.ascontiguousarray(np.swapaxes(out, -3, -2))


def _chunk_weight(w):
    """[256, 64] f32 -> [128, 2, 64] bf16."""
    bf = _bf16()
    wb = w.astype(bf)
    out = np.stack([wb[0:128], wb[128:256]], axis=1)
    return np.ascontiguousarray(out)


def make_in_maps(A_in_0, A_out_0, input_in, input_out,
                 W_head_in, W_head_out, a_in, a_out,
                 W_edge_in, W_edge_out, bias_iah, bias_oah):
    per = {
        "in": (A_in_0, input_in, W_head_in, W_edge_in, a_in, bias_iah),
        "out": (A_out_0, input_out, W_head_out, W_edge_out, a_out, bias_oah),
    }
    shared = {}
    shards = [dict() for _ in range(NCORES)]
    for br, (A, X, Wh, We, a, bv) in per.items():
        an = _chunk_rows(np.asarray(A, np.float32), pad_to=256)   # [B,128,2,256]
        at = _chunk_rows(np.transpose(np.asarray(A, np.float32), (0, 2, 1)))
        xt = _chunk_rows(np.transpose(np.asarray(X, np.float32), (0, 2, 1)),
                         pad_to=256)
        bsz = an.shape[0]
        packed = np.concatenate([an.reshape(bsz, 128, 512),
                                 at.reshape(bsz, 128, 400),
                                 xt.reshape(bsz, 128, 512)], axis=2)
        shared[f"WH_{br}"] = _chunk_weight(np.asarray(Wh, np.float32))
        shared[f"WE_{br}"] = _chunk_weight(np.asarray(We, np.float32))
        shared[f"AV_{br}"] = np.ascontiguousarray(np.concatenate([a, a]), dtype=np.float32)
        shared[f"BV_{br}"] = np.ascontiguousarray(bv, dtype=np.float32)
        for c in range(NCORES):
            s = slice(c * BPC, (c + 1) * BPC)
            shards[c][f"IN_{br}"] = np.ascontiguousarray(packed[s])
    for c in range(NCORES):
        shards[c].update(shared)
    return shards


def run(trace=False, **inputs):
    order = int(inputs.get("order", 3))
    nc = _get(order)
    in_maps = make_in_maps(
        A_in_0=inputs["A_in_0"], A_out_0=inputs["A_out_0"],
        input_in=inputs["input_in"], input_out=inputs["input_out"],
        W_head_in=inputs["W_head_in"], W_head_out=inputs["W_head_out"],
        a_in=inputs["a_in"], a_out=inputs["a_out"],
        W_edge_in=inputs["W_edge_in"], W_edge_out=inputs["W_edge_out"],
        bias_iah=inputs["bias_iah"], bias_oah=inputs["bias_oah"])
    kw2 = {}
    if trace:
        import os
        td = os.path.join(os.getcwd(), "trace_out")
        os.makedirs(td, exist_ok=True)
        kw2["tmpdir"] = td
    res = bass_utils.run_bass_kernel_spmd(nc, in_maps, core_ids=list(range(NCORES)),
                                          trace=trace, **kw2)
    out_in = np.concatenate(
        [res.results[c]["O_in"][:, 0:N, :] for c in range(NCORES)], axis=0)
    out_out = np.concatenate(
        [res.results[c]["O_out"][:, 0:N, :] for c in range(NCORES)], axis=0)
    return (out_in.astype(np.float32), out_out.astype(np.float32)), res


def kernel(**inputs):
    (out_in, out_out), _ = run(trace=False, **inputs)
    return out_in, out_out


# revision 19
# speedup vs baseline: 1.0205x; 1.0205x over previous
"""GAT cell (gnn_message_passing) Bass kernel for 8 Trainium2 NeuronCores.

Sharding: pure data parallelism over batch (64 graphs -> 8 per core), both
branches (in/out) on every core.

Host-side sharding also prepares layouts: bf16 cast (exact for the 0/1
adjacencies), row-chunking to the 128-partition grid, and the A^T / input^T
transposes, so the device does pure compute with large contiguous DMAs.

Math (per graph, per branch), done entirely in a TRANSPOSED layout so no
per-batch transposes of computed tensors are ever needed:
  x^T   = W_head^T @ input^T                      [att, N]
  xa^T  = a * x^T   (per-partition scale)
  s^T   = x @ (x*a)^T  via lhsT=x^T, rhs=xa^T     [N(j), N(i)]  == score^T
  B     = A^T;  B^k = (A^k)^T via lhsT=A (natural layout!)
  mask^T= binarize(B + B^2 + ... + B^order)       (exact in bf16: small ints)
  P^T   = exp(leakyrelu(s^T)) * mask^T            [j, i]
  Y     = input @ W_edge  via lhsT=input^T        [N(j), att]; augment ones col
  U     = P @ [Y | 1] via lhsT=P^T                [N(i), att+1]; col att = rowsum
  out   = U[:, :att] / (rowsum + eps) + bias
This equals softmax(where(mask, score, -1e12), axis=-1)*mask @ input @ W_edge
+ bias exactly (masked exps are exactly 0; all-masked rows give 0 rows).

PSUM bank trick for the reachability accumulator: B^2 matmuls write the bank,
the bank is evacuated to SBUF (rhs for B^3) while I@B re-adds and the B^3
matmuls keep accumulating into the same bank, so no separate I@B^2 pass.
"""

import numpy as np
from contextlib import ExitStack

import concourse.bass as bass
import concourse.bacc as bacc
import concourse.tile as tile
from concourse import mybir, bass_utils

F32, BF16 = mybir.dt.float32, mybir.dt.bfloat16
AF = mybir.ActivationFunctionType
ALU = mybir.AluOpType

NCORES = 8
B = 64
BPC = B // NCORES        # batches per core
N = 200                  # nodes per graph
H = 256                  # feature dim
ATT = 64                 # head dim
CH = [(0, 128), (1, 72)]  # (chunk index, rows) for the N=200 row split
EPS = 1e-20
BRS = ("in", "out")


def _make_identity(nc, identity):
    nc.gpsimd.memset(identity, 0.0)
    nc.gpsimd.affine_select(
        out=identity, in_=identity, compare_op=ALU.not_equal, fill=1.0,
        base=0, pattern=[[-1, 128]], channel_multiplier=1)


def _emit(ctx, tc, order, AN, AT, XT, WH, WE, AV, BV, O):
    nc = tc.nc
    consts = ctx.enter_context(tc.tile_pool(name="consts", bufs=1))
    pin = ctx.enter_context(tc.tile_pool(name="pin", bufs=3))
    pw = ctx.enter_context(tc.tile_pool(name="pw", bufs=6))
    pp1 = ctx.enter_context(tc.tile_pool(name="pp1", bufs=1, space="PSUM"))
    pp2 = ctx.enter_context(tc.tile_pool(name="pp2", bufs=2, space="PSUM"))

    ident = consts.tile([128, 128], BF16, tag="ident", name="ident")
    _make_identity(nc, ident)

    wh, we, av, bias = {}, {}, {}, {}
    for br in BRS:
        wh[br] = consts.tile([128, 2, ATT], BF16, tag=f"wh_{br}", name=f"wh_{br}")
        nc.sync.dma_start(out=wh[br], in_=WH[br])
        we[br] = consts.tile([128, 2, ATT], BF16, tag=f"we_{br}", name=f"we_{br}")
        nc.sync.dma_start(out=we[br], in_=WE[br])
        av[br] = consts.tile([128, 1], F32, tag=f"av_{br}", name=f"av_{br}")
        nc.sync.dma_start(out=av[br], in_=AV[br].rearrange("(a o) -> a o", o=1))
        bias[br] = consts.tile([128, ATT], F32, tag=f"bias_{br}", name=f"bias_{br}")
        bcast = bass.AP(tensor=BV[br].tensor, offset=BV[br].offset,
                        ap=[[0, 128], [1, ATT]])
        nc.gpsimd.dma_start(out=bias[br], in_=bcast)

    bufs = {}
    for qb in range(0, BPC, 4):
        for br in BRS:
            buf = pin.tile([128, 4, 1424], BF16, tag=f"buf_{br}",
                           name=f"buf_{br}")
            nc.sync.dma_start(out=buf, in_=AN[br][qb:qb + 4].rearrange(
                "b p f -> p b f"))
            bufs[br] = buf
        for pi in (0, 2):
          for br in BRS:
            pb = qb + pi
            buf = bufs[br]
            a0_, T_, iT_ = [], [], []
            for i in range(2):
                bi = pi + i
                a0_.append(buf[:, bi, 0:512].rearrange("p (c m) -> p c m", c=2))
                T_.append(buf[:, bi, 512:912].rearrange("p (c m) -> p c m", c=2))
                iT_.append(buf[:, bi, 912:1424].rearrange("p (c m) -> p c m",
                                                          c=2))

            # ---- x^T for both batches packed on 128 partitions ----
            # batch pb on partitions 0:64, batch pb+1 on 64:128; the W_head
            # stationary is shared, halving LDWEIGHTS.
            xt_ps = pp1.tile([128, 256], F32, tag="xt_ps", name="xt_ps")
            for i in range(2):
                for hc in range(2):
                    nc.tensor.matmul(xt_ps[i * 64:(i + 1) * 64, :],
                                     wh[br][:, hc, :], iT_[i][:, hc, :],
                                     start=(hc == 0), stop=(hc == 1))
            xt = pw.tile([128, 256], BF16, tag="xt", name="xt")
            nc.scalar.activation(out=xt, in_=xt_ps, func=AF.Copy)
            xa = pw.tile([128, 256], BF16, tag="xa", name="xa")
            nc.vector.tensor_scalar(out=xa, in0=xt, scalar1=av[br], scalar2=None,
                                    op0=ALU.mult)

            for i in range(2):
                b = pb + i
                a0, T, iT = a0_[i], T_[i], iT_[i]
                xtb = xt[i * 64:(i + 1) * 64, :]
                xab = xa[i * 64:(i + 1) * 64, :]

                # ---- score^T then exp(leaky(.)) ----
                sc_ps = pp2.tile([128, 2, N], F32, tag="sc_ps", name="sc_ps")
                for jc in range(2):
                    nc.tensor.matmul(sc_ps[:, jc, :],
                                     xtb[:, jc * 128:(jc + 1) * 128],
                                     xab[:, 0:N], start=True, stop=True)
                nc.scalar.activation(out=sc_ps, in_=sc_ps, func=AF.Prelu,
                                     alpha=0.2)
                es = pw.tile([128, 2, N], BF16, tag="es", name="es")
                nc.scalar.activation(out=es, in_=sc_ps, func=AF.Exp)

                # ---- reachability: B^2 bank, then B^3 bank = I@B^2 + B^3 ----
                b23 = None
                if order == 2:
                    b23 = pp2.tile([128, 2, N], F32, tag="b23", name="b23")
                    for mc in range(2):
                        for kc in range(2):
                            nc.tensor.matmul(b23[:, mc, :],
                                             a0[:, kc, mc * 128:(mc + 1) * 128],
                                             T[:, kc, :],
                                             start=(kc == 0), stop=(kc == 1))
                elif order >= 3:
                    assert order == 3, "only order<=3 supported"
                    b2_ps = pp1.tile([128, 2, N], F32, tag="b2_ps", name="b2_ps")
                    for mc in range(2):
                        for kc in range(2):
                            nc.tensor.matmul(b2_ps[:, mc, :],
                                             a0[:, kc, mc * 128:(mc + 1) * 128],
                                             T[:, kc, :],
                                             start=(kc == 0), stop=(kc == 1))
                    b2 = pw.tile([128, 2, N], BF16, tag="b2", name="b2")
                    nc.scalar.activation(out=b2, in_=b2_ps, func=AF.Copy)
                    b23 = pp2.tile([128, 2, N], F32, tag="b23", name="b23")
                    for mc in range(2):
                        nc.tensor.matmul(b23[:, mc, :], ident, b2[:, mc, :],
                                         start=True, stop=False)
                        for kc in range(2):
                            nc.tensor.matmul(b23[:, mc, :],
                                             a0[:, kc, mc * 128:(mc + 1) * 128],
                                             b2[:, kc, :],
                                             start=False, stop=(kc == 1))

                # ---- P^T = exp(leaky(s^T)) * max(bin(B^2+..), B), padded ----
                pt = pw.tile([128, 2, 256], BF16, tag="pt", name="pt")
                nc.gpsimd.memset(pt[:, :, N:256], 0.0)
                if order >= 2:
                    mk = pw.tile([128, 2, N], BF16, tag="mk", name="mk")
                    nc.vector.scalar_tensor_tensor(
                        out=mk, in0=b23, scalar=0.0, in1=T,
                        op0=ALU.is_gt, op1=ALU.max)
                else:
                    mk = T
                nc.vector.tensor_tensor(out=pt[:, :, 0:N], in0=es, in1=mk,
                                        op=ALU.mult)

                # ---- Y = input @ W_edge (+ ones column) ----
                y_ps = pp1.tile([128, 2, ATT + 1], F32, tag="y_ps", name="y_ps")
                for jc in range(2):
                    for hc in range(2):
                        nc.tensor.matmul(y_ps[:, jc, 0:ATT],
                                         iT[:, hc, jc * 128:(jc + 1) * 128],
                                         we[br][:, hc, :],
                                         start=(hc == 0), stop=(hc == 1))
                ys = pw.tile([128, 2, ATT + 1], BF16, tag="ys", name="ys")
                nc.scalar.activation(out=ys[:, :, 0:ATT], in_=y_ps[:, :, 0:ATT],
                                     func=AF.Copy)
                nc.gpsimd.memset(ys[:, :, ATT:ATT + 1], 1.0)

                # ---- U = P @ [Y|1] ; normalize + bias ----
                o_ps = pp1.tile([128, 2, ATT + 1], F32, tag="o_ps", name="o_ps")
                for ic in range(2):
                    for jc in range(2):
                        nc.tensor.matmul(o_ps[:, ic, :],
                                         pt[:, jc, ic * 128:(ic + 1) * 128],
                                         ys[:, jc, :],
                                         start=(jc == 0), stop=(jc == 1))
                if i == 0:
                    res_pair = pw.tile([128, 2, 2, ATT], F32, tag="res",
                                       name="res_pair")
                r = pw.tile([128, 2, 1], F32, tag="r", name="r")
                nc.vector.tensor_scalar(out=r, in0=o_ps[:, :, ATT:ATT + 1],
                                        scalar1=EPS, scalar2=None, op0=ALU.add)
                nc.vector.reciprocal(out=r, in_=r)
                for ic in range(2):
                    nc.vector.scalar_tensor_tensor(out=res_pair[:, i, ic, :],
                                                   in0=o_ps[:, ic, 0:ATT],
                                                   scalar=r[:, ic, 0:1],
                                                   in1=bias[br],
                                                   op0=ALU.mult, op1=ALU.add)
                if i == 1:
                    nc.gpsimd.dma_start(
                        out=O[br][pb:pb + 2].rearrange("b (c p) d -> p b c d",
                                                       c=2),
                        in_=res_pair)


def build(order: int) -> bacc.Bacc:
    nc = bacc.Bacc("TRN2", target_bir_lowering=False, debug=False,
                   enable_asserts=True, num_devices=NCORES)
    AN, AT, XT, WH, WE, AV, BV, O = {}, {}, {}, {}, {}, {}, {}, {}
    for br in BRS:
        AN[br] = nc.dram_tensor(f"IN_{br}", [BPC, 128, 1424], BF16,
                                kind="ExternalInput").ap()
        AT[br] = None
        XT[br] = None
        WH[br] = nc.dram_tensor(f"WH_{br}", [128, 2, ATT], BF16,
                                kind="ExternalInput").ap()
        WE[br] = nc.dram_tensor(f"WE_{br}", [128, 2, ATT], BF16,
                                kind="ExternalInput").ap()
        AV[br] = nc.dram_tensor(f"AV_{br}", [128], F32, kind="ExternalInput").ap()
        BV[br] = nc.dram_tensor(f"BV_{br}", [ATT], F32, kind="ExternalInput").ap()
        O[br] = nc.dram_tensor(f"O_{br}", [BPC, 256, ATT], F32,
                               kind="ExternalOutput").ap()
    with tile.TileContext(nc) as tc:
        with ExitStack() as ctx:
            _emit(ctx, tc, order, AN, AT, XT, WH, WE, AV, BV, O)
    nc.compile()
    return nc


_CACHE = {}


def _get(order: int) -> bacc.Bacc:
    if order not in _CACHE:
        _CACHE[order] = build(order)
    return _CACHE[order]


def _bf16():
    import ml_dtypes
    return ml_dtypes.bfloat16


def _chunk_rows(x, pad_to=None):
    """[..., R, C] f32 -> [..., 128, 2, Cp] bf16: rows chunked to the
    128-partition grid (zero rows 72..127 of chunk 1 when R==200) and the
    free dim optionally zero-padded to ``pad_to``."""
    bf = _bf16()
    lead = x.shape[:-2]
    r, c = x.shape[-2:]
    cp = pad_to or c
    out = np.zeros(lead + (2, 128, cp), dtype=bf)
    xb = x.astype(bf)
    out[..., 0, 0:128, 0:c] = xb[..., 0:128, :]
    out[..., 1, 0:r - 128, 0:c] = xb[..., 128:r, :]
    # reorder to [..., 128, 2, Cp]
    return np.ascontiguousarray(np.swapaxes(out, -3, -2))


def _chunk_weight(w):
    """[256, 64] f32 -> [128, 2, 64] bf16."""
    bf = _bf16()
    wb = w.astype(bf)
    out = np.stack([wb[0:128], wb[128:256]], axis=1)
    return np.ascontiguousarray(out)


def make_in_maps(A_in_0, A_out_0, input_in, input_out,
                 W_head_in, W_head_out, a_in, a_out,
                 W_edge_in, W_edge_out, bias_iah, bias_oah):
    per = {
        "in": (A_in_0, input_in, W_head_in, W_edge_in, a_in, bias_iah),
        "out": (A_out_0, input_out, W_head_out, W_edge_out, a_out, bias_oah),
    }
    shared = {}
    shards = [dict() for _ in range(NCORES)]
    for br, (A, X, Wh, We, a, bv) in per.items():
        an = _chunk_rows(np.asarray(A, np.float32), pad_to=256)   # [B,128,2,256]
        at = _chunk_rows(np.transpose(np.asarray(A, np.float32), (0, 2, 1)))
        xt = _chunk_rows(np.transpose(np.asarray(X, np.float32), (0, 2, 1)),
                         pad_to=256)
        bsz = an.shape[0]
        packed = np.concatenate([an.reshape(bsz, 128, 512),
                                 at.reshape(bsz, 128, 400),
                                 xt.reshape(bsz, 128, 512)], axis=2)
        shared[f"WH_{br}"] = _chunk_weight(np.asarray(Wh, np.float32))
        shared[f"WE_{br}"] = _chunk_weight(np.asarray(We, np.float32))
        shared[f"AV_{br}"] = np.ascontiguousarray(np.concatenate([a, a]), dtype=np.float32)
        shared[f"BV_{br}"] = np.ascontiguousarray(bv, dtype=np.float32)
        for c in range(NCORES):
            s = slice(c * BPC, (c + 1) * BPC)
            shards[c][f"IN_{br}"] = np.ascontiguousarray(packed[s])
    for c in range(NCORES):
        shards[c].update(shared)
    return shards


def run(trace=False, **inputs):
    order = int(inputs.get("order", 3))
    nc = _get(order)
    in_maps = make_in_maps(
        A_in_0=inputs["A_in_0"], A_out_0=inputs["A_out_0"],
        input_in=inputs["input_in"], input_out=inputs["input_out"],
        W_head_in=inputs["W_head_in"], W_head_out=inputs["W_head_out"],
        a_in=inputs["a_in"], a_out=inputs["a_out"],
        W_edge_in=inputs["W_edge_in"], W_edge_out=inputs["W_edge_out"],
        bias_iah=inputs["bias_iah"], bias_oah=inputs["bias_oah"])
    kw2 = {}
    if trace:
        import os
        td = os.path.join(os.getcwd(), "trace_out")
        os.makedirs(td, exist_ok=True)
        kw2["tmpdir"] = td
    res = bass_utils.run_bass_kernel_spmd(nc, in_maps, core_ids=list(range(NCORES)),
                                          trace=trace, **kw2)
    out_in = np.concatenate(
        [res.results[c]["O_in"][:, 0:N, :] for c in range(NCORES)], axis=0)
    out_out = np.concatenate(
        [res.results[c]["O_out"][:, 0:N, :] for c in range(NCORES)], axis=0)
    return (out_in.astype(np.float32), out_out.astype(np.float32)), res


def kernel(**inputs):
    (out_in, out_out), _ = run(trace=False, **inputs)
    return out_in, out_out


# revision 20
# speedup vs baseline: 1.0298x; 1.0092x over previous
"""GAT cell (gnn_message_passing) Bass kernel for 8 Trainium2 NeuronCores.

Sharding: pure data parallelism over batch (64 graphs -> 8 per core), both
branches (in/out) on every core.

Host-side sharding also prepares layouts: bf16 cast (exact for the 0/1
adjacencies), row-chunking to the 128-partition grid, and the A^T / input^T
transposes, so the device does pure compute with large contiguous DMAs.

Math (per graph, per branch), done entirely in a TRANSPOSED layout so no
per-batch transposes of computed tensors are ever needed:
  x^T   = W_head^T @ input^T                      [att, N]
  xa^T  = a * x^T   (per-partition scale)
  s^T   = x @ (x*a)^T  via lhsT=x^T, rhs=xa^T     [N(j), N(i)]  == score^T
  B     = A^T;  B^k = (A^k)^T via lhsT=A (natural layout!)
  mask^T= binarize(B + B^2 + ... + B^order)       (exact in bf16: small ints)
  P^T   = exp(leakyrelu(s^T)) * mask^T            [j, i]
  Y     = input @ W_edge  via lhsT=input^T        [N(j), att]; augment ones col
  U     = P @ [Y | 1] via lhsT=P^T                [N(i), att+1]; col att = rowsum
  out   = U[:, :att] / (rowsum + eps) + bias
This equals softmax(where(mask, score, -1e12), axis=-1)*mask @ input @ W_edge
+ bias exactly (masked exps are exactly 0; all-masked rows give 0 rows).

PSUM bank trick for the reachability accumulator: B^2 matmuls write the bank,
the bank is evacuated to SBUF (rhs for B^3) while I@B re-adds and the B^3
matmuls keep accumulating into the same bank, so no separate I@B^2 pass.
"""

import numpy as np
from contextlib import ExitStack

import concourse.bass as bass
import concourse.bacc as bacc
import concourse.tile as tile
from concourse import mybir, bass_utils

F32, BF16 = mybir.dt.float32, mybir.dt.bfloat16
AF = mybir.ActivationFunctionType
ALU = mybir.AluOpType

NCORES = 8
B = 64
BPC = B // NCORES        # batches per core
N = 200                  # nodes per graph
H = 256                  # feature dim
ATT = 64                 # head dim
CH = [(0, 128), (1, 72)]  # (chunk index, rows) for the N=200 row split
EPS = 1e-20
BRS = ("in", "out")


def _make_identity(nc, identity):
    nc.gpsimd.memset(identity, 0.0)
    nc.gpsimd.affine_select(
        out=identity, in_=identity, compare_op=ALU.not_equal, fill=1.0,
        base=0, pattern=[[-1, 128]], channel_multiplier=1)


def _emit(ctx, tc, order, AN, AT, XT, WH, WE, AV, BV, O):
    nc = tc.nc
    consts = ctx.enter_context(tc.tile_pool(name="consts", bufs=1))
    pin = ctx.enter_context(tc.tile_pool(name="pin", bufs=2))
    pw = ctx.enter_context(tc.tile_pool(name="pw", bufs=8))
    pp1 = ctx.enter_context(tc.tile_pool(name="pp1", bufs=1, space="PSUM"))
    pp2 = pp1

    ident = consts.tile([128, 128], BF16, tag="ident", name="ident")
    _make_identity(nc, ident)

    wh, we, av, bias = {}, {}, {}, {}
    for br in BRS:
        wh[br] = consts.tile([128, 2, ATT], BF16, tag=f"wh_{br}", name=f"wh_{br}")
        nc.sync.dma_start(out=wh[br], in_=WH[br])
        we[br] = consts.tile([128, 2, ATT], BF16, tag=f"we_{br}", name=f"we_{br}")
        nc.sync.dma_start(out=we[br], in_=WE[br])
        av[br] = consts.tile([128, 1], F32, tag=f"av_{br}", name=f"av_{br}")
        nc.sync.dma_start(out=av[br], in_=AV[br].rearrange("(a o) -> a o", o=1))
        bias[br] = consts.tile([128, ATT], F32, tag=f"bias_{br}", name=f"bias_{br}")
        bcast = bass.AP(tensor=BV[br].tensor, offset=BV[br].offset,
                        ap=[[0, 128], [1, ATT]])
        nc.gpsimd.dma_start(out=bias[br], in_=bcast)

    bufs = {}
    for qb in range(0, BPC, 4):
        for br in BRS:
            buf = pin.tile([128, 4, 1424], BF16, tag=f"buf_{br}",
                           name=f"buf_{br}")
            nc.sync.dma_start(out=buf, in_=AN[br][qb:qb + 4].rearrange(
                "b p f -> p b f"))
            bufs[br] = buf
        for pi in (0, 2):
          for br in BRS:
            pb = qb + pi
            buf = bufs[br]
            a0_, T_, iT_ = [], [], []
            for i in range(2):
                bi = pi + i
                a0_.append(buf[:, bi, 0:512].rearrange("p (c m) -> p c m", c=2))
                T_.append(buf[:, bi, 512:912].rearrange("p (c m) -> p c m", c=2))
                iT_.append(buf[:, bi, 912:1424].rearrange("p (c m) -> p c m",
                                                          c=2))

            # ---- x^T for both batches packed on 128 partitions ----
            # batch pb on partitions 0:64, batch pb+1 on 64:128; the W_head
            # stationary is shared, halving LDWEIGHTS.
            xt_ps = pp1.tile([128, 256], F32, tag="xt_ps", name="xt_ps")
            for i in range(2):
                for hc in range(2):
                    nc.tensor.matmul(xt_ps[i * 64:(i + 1) * 64, :],
                                     wh[br][:, hc, :], iT_[i][:, hc, :],
                                     start=(hc == 0), stop=(hc == 1))
            xt = pw.tile([128, 256], BF16, tag="xt", name="xt")
            nc.scalar.activation(out=xt, in_=xt_ps, func=AF.Copy)
            xa = pw.tile([128, 256], BF16, tag="xa", name="xa")
            nc.vector.tensor_scalar(out=xa, in0=xt, scalar1=av[br], scalar2=None,
                                    op0=ALU.mult)

            for i in range(2):
                b = pb + i
                a0, T, iT = a0_[i], T_[i], iT_[i]
                xtb = xt[i * 64:(i + 1) * 64, :]
                xab = xa[i * 64:(i + 1) * 64, :]

                # ---- score^T then exp(leaky(.)) ----
                sc_ps = pp2.tile([128, 2, N], F32, tag="sc_ps", name="sc_ps", bufs=2)
                for jc in range(2):
                    nc.tensor.matmul(sc_ps[:, jc, :],
                                     xtb[:, jc * 128:(jc + 1) * 128],
                                     xab[:, 0:N], start=True, stop=True)
                nc.scalar.activation(out=sc_ps, in_=sc_ps, func=AF.Prelu,
                                     alpha=0.2)
                es = pw.tile([128, 2, N], BF16, tag="es", name="es")
                nc.scalar.activation(out=es, in_=sc_ps, func=AF.Exp)

                # ---- reachability: B^2 bank, then B^3 bank = I@B^2 + B^3 ----
                b23 = None
                if order == 2:
                    b23 = pp2.tile([128, 2, N], F32, tag="b23", name="b23", bufs=2)
                    for mc in range(2):
                        for kc in range(2):
                            nc.tensor.matmul(b23[:, mc, :],
                                             a0[:, kc, mc * 128:(mc + 1) * 128],
                                             T[:, kc, :],
                                             start=(kc == 0), stop=(kc == 1))
                elif order >= 3:
                    assert order == 3, "only order<=3 supported"
                    b2_ps = pp1.tile([128, 2, N], F32, tag="b2_ps", name="b2_ps")
                    for mc in range(2):
                        for kc in range(2):
                            nc.tensor.matmul(b2_ps[:, mc, :],
                                             a0[:, kc, mc * 128:(mc + 1) * 128],
                                             T[:, kc, :],
                                             start=(kc == 0), stop=(kc == 1))
                    b2 = pw.tile([128, 2, N], BF16, tag="b2", name="b2")
                    nc.scalar.activation(out=b2, in_=b2_ps, func=AF.Copy)
                    b23 = pp2.tile([128, 2, N], F32, tag="b23", name="b23", bufs=2)
                    for mc in range(2):
                        nc.tensor.matmul(b23[:, mc, :], ident, b2[:, mc, :],
                                         start=True, stop=False)
                        for kc in range(2):
                            nc.tensor.matmul(b23[:, mc, :],
                                             a0[:, kc, mc * 128:(mc + 1) * 128],
                                             b2[:, kc, :],
                                             start=False, stop=(kc == 1))

                # ---- P^T = exp(leaky(s^T)) * max(bin(B^2+..), B), padded ----
                pt = pw.tile([128, 2, 256], BF16, tag="pt", name="pt")
                nc.gpsimd.memset(pt[:, :, N:256], 0.0)
                if order >= 2:
                    mk = pw.tile([128, 2, N], BF16, tag="mk", name="mk")
                    nc.vector.scalar_tensor_tensor(
                        out=mk, in0=b23, scalar=0.0, in1=T,
                        op0=ALU.is_gt, op1=ALU.max)
                else:
                    mk = T
                nc.vector.tensor_tensor(out=pt[:, :, 0:N], in0=es, in1=mk,
                                        op=ALU.mult)

                # ---- Y = input @ W_edge (+ ones column) ----
                yo_ps = pp1.tile([128, 2, 2, ATT + 1], F32, tag="yo_ps",
                                 name="yo_ps", bufs=2)
                for jc in range(2):
                    for hc in range(2):
                        nc.tensor.matmul(yo_ps[:, 0, jc, 0:ATT],
                                         iT[:, hc, jc * 128:(jc + 1) * 128],
                                         we[br][:, hc, :],
                                         start=(hc == 0), stop=(hc == 1))
                ys = pw.tile([128, 2, ATT + 1], BF16, tag="ys", name="ys")
                nc.vector.tensor_copy(ys[:, :, 0:ATT], yo_ps[:, 0, :, 0:ATT])
                nc.gpsimd.memset(ys[:, :, ATT:ATT + 1], 1.0)

                # ---- U = P @ [Y|1] ; normalize + bias ----
                o_ps = yo_ps[:, 1, :, :]
                for ic in range(2):
                    for jc in range(2):
                        nc.tensor.matmul(o_ps[:, ic, :],
                                         pt[:, jc, ic * 128:(ic + 1) * 128],
                                         ys[:, jc, :],
                                         start=(jc == 0), stop=(jc == 1))
                if i == 0:
                    res_pair = pw.tile([128, 2, 2, ATT], F32, tag="res",
                                       name="res_pair")
                r = pw.tile([128, 2, 1], F32, tag="r", name="r")
                nc.vector.tensor_scalar(out=r, in0=o_ps[:, :, ATT:ATT + 1],
                                        scalar1=EPS, scalar2=None, op0=ALU.add)
                nc.vector.reciprocal(out=r, in_=r)
                for ic in range(2):
                    nc.vector.scalar_tensor_tensor(out=res_pair[:, i, ic, :],
                                                   in0=o_ps[:, ic, 0:ATT],
                                                   scalar=r[:, ic, 0:1],
                                                   in1=bias[br],
                                                   op0=ALU.mult, op1=ALU.add)
                if i == 1:
                    nc.gpsimd.dma_start(
                        out=O[br][pb:pb + 2].rearrange("b (c p) d -> p b c d",
                                                       c=2),
                        in_=res_pair)


def build(order: int) -> bacc.Bacc:
    nc = bacc.Bacc("TRN2", target_bir_lowering=False, debug=False,
                   enable_asserts=True, num_devices=NCORES)
    AN, AT, XT, WH, WE, AV, BV, O = {}, {}, {}, {}, {}, {}, {}, {}
    for br in BRS:
        AN[br] = nc.dram_tensor(f"IN_{br}", [BPC, 128, 1424], BF16,
                                kind="ExternalInput").ap()
        AT[br] = None
        XT[br] = None
        WH[br] = nc.dram_tensor(f"WH_{br}", [128, 2, ATT], BF16,
                                kind="ExternalInput").ap()
        WE[br] = nc.dram_tensor(f"WE_{br}", [128, 2, ATT], BF16,
                                kind="ExternalInput").ap()
        AV[br] = nc.dram_tensor(f"AV_{br}", [128], F32, kind="ExternalInput").ap()
        BV[br] = nc.dram_tensor(f"BV_{br}", [ATT], F32, kind="ExternalInput").ap()
        O[br] = nc.dram_tensor(f"O_{br}", [BPC, 256, ATT], F32,
                               kind="ExternalOutput").ap()
    with tile.TileContext(nc) as tc:
        with ExitStack() as ctx:
            _emit(ctx, tc, order, AN, AT, XT, WH, WE, AV, BV, O)
    nc.compile()
    return nc


_CACHE = {}


def _get(order: int) -> bacc.Bacc:
    if order not in _CACHE:
        _CACHE[order] = build(order)
    return _CACHE[order]


def _bf16():
    import ml_dtypes
    return ml_dtypes.bfloat16


def _chunk_rows(x, pad_to=None):
    """[..., R, C] f32 -> [..., 128, 2, Cp] bf16: rows chunked to the
    128-partition grid (zero rows 72..127 of chunk 1 when R==200) and the
    free dim optionally zero-padded to ``pad_to``."""
    bf = _bf16()
    lead = x.shape[:-2]
    r, c = x.shape[-2:]
    cp = pad_to or c
    out = np.zeros(lead + (2, 128, cp), dtype=bf)
    xb = x.astype(bf)
    out[..., 0, 0:128, 0:c] = xb[..., 0:128, :]
    out[..., 1, 0:r - 128, 0:c] = xb[..., 128:r, :]
    # reorder to [..., 128, 2, Cp]
    return np.ascontiguousarray(np.swapaxes(out, -3, -2))


def _chunk_weight(w):
    """[256, 64] f32 -> [128, 2, 64] bf16."""
    bf = _bf16()
    wb = w.astype(bf)
    out = np.stack([wb[0:128], wb[128:256]], axis=1)
    return np.ascontiguousarray(out)


def make_in_maps(A_in_0, A_out_0, input_in, input_out,
                 W_head_in, W_head_out, a_in, a_out,
                 W_edge_in, W_edge_out, bias_iah, bias_oah):
    per = {
        "in": (A_in_0, input_in, W_head_in, W_edge_in, a_in, bias_iah),
        "out": (A_out_0, input_out, W_head_out, W_edge_out, a_out, bias_oah),
    }
    shared = {}
    shards = [dict() for _ in range(NCORES)]
    for br, (A, X, Wh, We, a, bv) in per.items():
        an = _chunk_rows(np.asarray(A, np.float32), pad_to=256)   # [B,128,2,256]
        at = _chunk_rows(np.transpose(np.asarray(A, np.float32), (0, 2, 1)))
        xt = _chunk_rows(np.transpose(np.asarray(X, np.float32), (0, 2, 1)),
                         pad_to=256)
        bsz = an.shape[0]
        packed = np.concatenate([an.reshape(bsz, 128, 512),
                                 at.reshape(bsz, 128, 400),
                                 xt.reshape(bsz, 128, 512)], axis=2)
        shared[f"WH_{br}"] = _chunk_weight(np.asarray(Wh, np.float32))
        shared[f"WE_{br}"] = _chunk_weight(np.asarray(We, np.float32))
        shared[f"AV_{br}"] = np.ascontiguousarray(np.concatenate([a, a]), dtype=np.float32)
        shared[f"BV_{br}"] = np.ascontiguousarray(bv, dtype=np.float32)
        for c in range(NCORES):
            s = slice(c * BPC, (c + 1) * BPC)
            shards[c][f"IN_{br}"] = np.ascontiguousarray(packed[s])
    for c in range(NCORES):
        shards[c].update(shared)
    return shards


def run(trace=False, **inputs):
    order = int(inputs.get("order", 3))
    nc = _get(order)
    in_maps = make_in_maps(
        A_in_0=inputs["A_in_0"], A_out_0=inputs["A_out_0"],
        input_in=inputs["input_in"], input_out=inputs["input_out"],
        W_head_in=inputs["W_head_in"], W_head_out=inputs["W_head_out"],
        a_in=inputs["a_in"], a_out=inputs["a_out"],
        W_edge_in=inputs["W_edge_in"], W_edge_out=inputs["W_edge_out"],
        bias_iah=inputs["bias_iah"], bias_oah=inputs["bias_oah"])
    kw2 = {}
    if trace:
        import os
        td = os.path.join(os.getcwd(), "trace_out")
        os.makedirs(td, exist_ok=True)
        kw2["tmpdir"] = td
    res = bass_utils.run_bass_kernel_spmd(nc, in_maps, core_ids=list(range(NCORES)),
                                          trace=trace, **kw2)
    out_in = np.concatenate(
        [res.results[c]["O_in"][:, 0:N, :] for c in range(NCORES)], axis=0)
    out_out = np.concatenate(
        [res.results[c]["O_out"][:, 0:N, :] for c in range(NCORES)], axis=0)
    return (out_in.astype(np.float32), out_out.astype(np.float32)), res


def kernel(**inputs):
    (out_in, out_out), _ = run(trace=False, **inputs)
    return out_in, out_out
